# revision 2
# baseline (speedup 1.0000x reference)
"""Trainium2 Bass kernel for DualThresholdSelfregulatingIntegrate.

Computes, bit-matched to the jax reference:
    rates  = relu(x) * DT                     # [B, T, D]
    c      = init[:, None, :] + cumsum(rates, axis=1)
    spikes = floor(c) - floor(c_prev)
    out    = spikes / DT

Strategy (per core, pure data-parallel over batch):
  - natural-layout loads [t, d]; relu on GPSIMD
  - cumsum over each 128-step time chunk via one fp32 PE matmul per
    (chunk, d-block): out[d, t'] = sum_t rates[t, d] * U[t, t']  with
    U upper-triangular ones -- the matmul computes the chunk cumsum AND
    the [t,d]->[d,t] transpose in one op, at near-fp64 accuracy
  - carries across chunks kept per (d-partition, chunk) and folded into a
    single DVE scalar_tensor_tensor: Gi2 = rint(2*c_local + (2*carr-0.5))
    with int16 output; floor = Gi2 >> 1 realized as (Gi2 & -2) and a
    final scale of K/2, which keeps every rounding tie exactly on the
    true floor boundary
  - spike diff along the free (time) axis in int16, transposed back to
    natural layout via PE int16 transposes, scaled to f32 on ScalarE
"""

import sys

sys.path.insert(0, "/opt/trn_rl_repo")

import numpy as np

import concourse.bass as bass  # noqa: F401  (registers engines)
import concourse.tile as tile
from concourse import bacc, mybir
from concourse.bass_utils import run_bass_kernel_spmd

N_CORES = 8
B, T, D = 16, 2048, 1024
BC = B // N_CORES          # batches per core
CH = 128                   # time-chunk (matmul contraction) size
NCH = T // CH              # 16 chunks
NDB = D // CH              # 8 d-blocks
dt = mybir.dt

K1000 = float(np.float32(1.0) / np.float32(0.001))   # matches reference's /DT
K500 = float(np.float32(K1000) / np.float32(2.0))    # exact half

_cache = {}


def build_nc():
    nc = bacc.Bacc("TRN2", target_bir_lowering=False, debug=False)
    x = nc.dram_tensor("x", [BC, T, D], dt.float32, kind="ExternalInput")
    v0t = nc.dram_tensor("v0t", [BC, CH, NDB], dt.float32, kind="ExternalInput")
    u = nc.dram_tensor("u", [CH, CH], dt.float32, kind="ExternalInput")
    ident = nc.dram_tensor("ident", [CH, CH], dt.float16, kind="ExternalInput")
    y = nc.dram_tensor("y", [BC, T, D], dt.float32, kind="ExternalOutput")

    AL = mybir.AluOpType
    AF = mybir.ActivationFunctionType
    EW = 130  # E-tile row stride: boundary col + 128 data cols + pad (even)

    with tile.TileContext(nc) as tc:
        with tc.tile_pool(name="xin", bufs=4) as xin_p, \
             tc.tile_pool(name="rates", bufs=4) as rates_p, \
             tc.tile_pool(name="gi", bufs=3) as gi_p, \
             tc.tile_pool(name="ee", bufs=3) as e_p, \
             tc.tile_pool(name="dd", bufs=3) as d_p, \
             tc.tile_pool(name="oo", bufs=4) as o_p, \
             tc.tile_pool(name="small", bufs=4) as sm_p, \
             tc.tile_pool(name="consts", bufs=1) as c_p, \
             tc.tile_pool(name="pc", bufs=3, space="PSUM") as pc_p, \
             tc.tile_pool(name="po", bufs=2, space="PSUM") as po_p:

            ut = c_p.tile([CH, CH], dt.float32, tag="ut")
            nc.sync.dma_start(ut[:], u[:])
            it = c_p.tile([CH, CH], dt.float16, tag="it")
            nc.sync.dma_start(it[:], ident[:])
            v0tt = c_p.tile([CH, BC * NDB], dt.float32, tag="v0tt")
            nc.sync.dma_start(
                v0tt[:].rearrange("p (b j) -> p b j", b=BC),
                v0t[:].rearrange("b p j -> p b j"),
            )
            v03 = v0tt[:].rearrange("p (b j) -> p b j", b=BC)

            for b in range(BC):
                carr_old = None
                e_prev = None
                for k in range(NCH):
                    xk = xin_p.tile([CH, D], dt.float32, tag="xk")
                    nc.sync.dma_start(xk[:], x[b, k * CH:(k + 1) * CH, :])
                    rk = rates_p.tile([CH, D], dt.float32, tag="rk")
                    nc.gpsimd.tensor_scalar(rk[:], xk[:], 0.001, 0.0,
                                            op0=AL.mult, op1=AL.max)

                    pck = pc_p.tile([CH, D], dt.float32, tag="pck")
                    for j in range(NDB):
                        nc.tensor.matmul(pck[:, j * CH:(j + 1) * CH],
                                         rk[:, j * CH:(j + 1) * CH], ut[:],
                                         start=True, stop=True)
                    pc3 = pck[:].rearrange("p (j t) -> p j t", j=NDB)

                    # carry -> 2*carr - 0.5 (both steps exact in fp32)
                    carr2m = sm_p.tile([CH, NDB], dt.float32, tag="c2m")
                    if carr_old is None:
                        nc.vector.tensor_scalar(carr2m[:], v03[:, b, :], 2.0, -0.5,
                                                op0=AL.mult, op1=AL.add)
                    else:
                        nc.vector.tensor_scalar(carr2m[:], carr_old[:], 2.0, -0.5,
                                                op0=AL.mult, op1=AL.add)

                    # Gi2 = rint(2*c_local + (2*carr-0.5)) as int16
                    gik = gi_p.tile([CH, NDB * CH], dt.int16, tag="gik")
                    g3 = gik[:].rearrange("p (j t) -> p j t", j=NDB)
                    cb = carr2m[:].unsqueeze(2).broadcast_to([CH, NDB, CH])
                    nc.vector.scalar_tensor_tensor(g3, pc3, 2.0, cb,
                                                   op0=AL.mult, op1=AL.add)

                    # E = Gi2 & -2  (= 2*floor), with per-dblk boundary col
                    ek = e_p.tile([CH, NDB * EW], dt.int16, tag="ek")
                    e3 = ek[:].rearrange("p (j t) -> p j t", j=NDB)
                    nc.vector.tensor_scalar(e3[:, :, 1:CH + 1], g3, -2, None,
                                            op0=AL.bitwise_and)
                    if e_prev is None:
                        nc.vector.memset(e3[:, :, 0], 0)
                    else:
                        nc.vector.tensor_copy(e3[:, :, 0], e_prev[:, :, CH])

                    # D = E_t - E_{t-1}  (int16; values 0 or 2)
                    dk = d_p.tile([CH, NDB * CH], dt.float16, tag="dk")
                    d3 = dk[:].rearrange("p (j t) -> p j t", j=NDB)
                    nc.vector.tensor_tensor(d3, e3[:, :, 1:CH + 1], e3[:, :, 0:CH],
                                            op=AL.subtract)

                    # transpose back to natural [t, d] and scale to f32
                    pok = po_p.tile([CH, D], dt.float16, tag="pok")
                    for j in range(NDB):
                        nc.tensor.transpose(pok[:, j * CH:(j + 1) * CH],
                                            dk[:, j * CH:(j + 1) * CH], it[:])
                    yo = o_p.tile([CH, D], dt.float32, tag="yo")
                    nc.scalar.activation(yo[:], pok[:], AF.Copy, bias=0.0, scale=K500)
                    nc.sync.dma_start(y[b, k * CH:(k + 1) * CH, :], yo[:])

                    # carry update: carr += chunk totals (psum col 127)
                    carr_new = sm_p.tile([CH, NDB], dt.float32, tag="carr")
                    if carr_old is None:
                        nc.vector.tensor_tensor(carr_new[:], v03[:, b, :],
                                                pc3[:, :, CH - 1], op=AL.add)
                    else:
                        nc.vector.tensor_tensor(carr_new[:], carr_old[:],
                                                pc3[:, :, CH - 1], op=AL.add)
                    carr_old = carr_new
                    e_prev = e3
    nc.compile()
    return nc


def _get_nc():
    if "nc" not in _cache:
        _cache["nc"] = build_nc()
    return _cache["nc"]


def _make_in_maps(x, v0):
    uv = np.triu(np.ones((CH, CH), dtype=np.float32))
    iv = np.eye(CH, dtype=np.float16)
    in_maps = []
    for c in range(N_CORES):
        xb = np.ascontiguousarray(x[BC * c:BC * (c + 1)])
        v0b = v0[BC * c:BC * (c + 1)]
        v0tb = np.ascontiguousarray(
            v0b.reshape(BC, NDB, CH).transpose(0, 2, 1).astype(np.float32))
        in_maps.append({"x": xb, "v0t": v0tb, "u": uv, "ident": iv})
    return in_maps


def kernel(inputs, initial_state):
    x = np.ascontiguousarray(np.asarray(inputs, dtype=np.float32))
    v0 = np.ascontiguousarray(np.asarray(initial_state, dtype=np.float32))
    assert x.shape == (B, T, D) and v0.shape == (B, D)
    nc = _get_nc()
    res = run_bass_kernel_spmd(nc, _make_in_maps(x, v0),
                               core_ids=list(range(N_CORES)))
    out = np.empty((B, T, D), dtype=np.float32)
    for c in range(N_CORES):
        out[BC * c:BC * (c + 1)] = res.results[c]["y"]
    return out
